# revision 1
# baseline (speedup 1.0000x reference)
"""AudioAttentionPooler Trainium2 kernel.

Algorithm (algebraically identical to the reference, ~60x fewer FLOPs):
  scores[b,t,h] = x[b,t,:] @ Wq[:,h]        Wq = fold(query*scale, kv_w_k)  [C,h]
  (k-bias shifts scores uniformly along t -> softmax-invariant -> dropped)
  e = exp(scores)                           (mask folded into x and Z instead)
  Z[b,h] = sum_t e[b,t,h] * mask[b,t]
  px[b,h,:] = sum_t e[b,t,h] * (mask[b,t] * x[b,t,:])   (pool BEFORE v-proj)
  out1[b,h*64+d] = (px[b,h,:] @ Wv[:,h*64+d]) / Z[b,h]
  out = out1 @ out_w + (kv_b_v @ out_w + out_b)   (v-bias exact: attn sums to 1)

Sharding: data-parallel over batch, 4 batch elements per core x 8 cores.
x is fed in both [T,C] and [C,T] layouts (host transpose) because the PE
contracts over the partition dim: scores contract over C, pooling over T.
"""

import numpy as np
import ml_dtypes

BF16 = ml_dtypes.bfloat16

HIDDEN = 1024
NH = 16
HD = 64
PROJ = 1024
B, T = 32, 2048
NCORES = 8
NB = B // NCORES          # 4 batch elems per core
KT = HIDDEN // 128        # 8 C-tiles
MT = T // 128             # 16 T-chunks
F8 = ml_dtypes.float8_e4m3
F8MAX = 240.0             # conservative e4m3 range cap

_CACHED_NC = None


def _build_nc(reps=1):
    import concourse.bacc as bacc
    import concourse.mybir as mybir
    import concourse.tile as tile

    f32 = mybir.dt.float32
    bf16 = mybir.dt.bfloat16
    f8 = mybir.dt.float8e4

    nc = bacc.Bacc("TRN2", target_bir_lowering=False, debug=False)

    x_d = nc.dram_tensor("x", [NB, T, HIDDEN], bf16, kind="ExternalInput")
    xt_d = nc.dram_tensor("xt", [NB, 128, MT, KT, 128], f8, kind="ExternalInput")
    wq_d = nc.dram_tensor("wq", [128, KT, NH], f8, kind="ExternalInput")
    wv_d = nc.dram_tensor("wv", [128, KT, NH, HD], bf16, kind="ExternalInput")
    wo_d = nc.dram_tensor("wo", [128, KT, 2, 512], bf16, kind="ExternalInput")
    mcol_d = nc.dram_tensor("mcol", [128, NB, MT], bf16, kind="ExternalInput")
    biasrep_d = nc.dram_tensor("biasrep", [NB, PROJ], f32, kind="ExternalInput")
    onescol_d = nc.dram_tensor("onescol", [1, 128], f32, kind="ExternalInput")
    idf_d = nc.dram_tensor("idf", [128, 128], f32, kind="ExternalInput")
    escale_d = nc.dram_tensor("escale", [128, 1], f32, kind="ExternalInput")
    out_d = nc.dram_tensor("out", [NB, PROJ], f32, kind="ExternalOutput")

    from contextlib import nullcontext

    with tile.TileContext(nc) as tc:
        with (
            tc.tile_pool(name="consts", bufs=1) as consts,
            tc.tile_pool(name="xpool", bufs=3) as xpool,
            tc.tile_pool(name="xtpool", bufs=3) as xtpool,
            tc.tile_pool(name="work", bufs=3) as work,
            tc.tile_pool(name="small", bufs=1) as small,
            tc.tile_pool(name="scps", bufs=2, space="PSUM") as scps,
            tc.tile_pool(name="pxps", bufs=2, space="PSUM") as pxps,
            tc.tile_pool(name="tps", bufs=2, space="PSUM") as tps,
            tc.tile_pool(name="bigps", bufs=1, space="PSUM") as bigps,
        ):
            wq_sb = consts.tile([128, KT, NH], f8)
            wv_sb = consts.tile([128, KT, NH, HD], bf16)
            wo_sb = consts.tile([128, KT, 2, 512], bf16)
            mcol_sb = consts.tile([128, NB, MT], bf16)
            biasrep_sb = consts.tile([NB, PROJ], f32)
            onescol_sb = consts.tile([1, 128], f32)
            idf_sb = consts.tile([128, 128], f32)
            escale_sb = consts.tile([128, 1], f32)
            nc.sync.dma_start(wq_sb[:], wq_d[:])

            # persistent accumulators across the b-loop
            pxall_sb = small.tile([128, KT, NH, NB], bf16)

            rep_ctx = tc.For_i(0, reps, 1) if reps > 1 else nullcontext()
            with rep_ctx:
              for b in range(NB):
                  x_sb = xpool.tile([128, MT, HIDDEN], bf16)
                  xt_sb = xtpool.tile([128, MT, KT, 128], f8)
                  for m4 in range(4):
                      nc.sync.dma_start(
                          xt_sb[:, m4 * 4:(m4 + 1) * 4],
                          xt_d[b, :, m4 * 4:(m4 + 1) * 4],
                      )
                  for m4 in range(4):
                      nc.sync.dma_start(
                          x_sb[:, m4 * 4:(m4 + 1) * 4],
                          x_d[b, m4 * 512:(m4 + 1) * 512].rearrange(
                              "(m p) c -> p m c", p=128
                          ),
                      )
                  # deferred const loads, ordered by first use so early DMA
                  # bandwidth goes to the batch data stream; stage-3/4 weights
                  # stream per-k AFTER all batch data so the pooling loop is
                  # never delayed and stage 3/4 chase the weight chunks
                  if b == 0:
                      nc.sync.dma_start(escale_sb[:], escale_d[:])
                      nc.sync.dma_start(onescol_sb[:], onescol_d[:])
                      nc.sync.dma_start(mcol_sb[:], mcol_d[:])
                      nc.sync.dma_start(idf_sb[:], idf_d[:])
                  elif b == NB - 1:
                      for k in range(KT):
                          nc.sync.dma_start(wv_sb[:, k], wv_d[:, k])
                      for k in range(KT):
                          nc.sync.dma_start(wo_sb[:, k], wo_d[:, k])
                      nc.sync.dma_start(biasrep_sb[:], biasrep_d[:])

                  # --- scores[t, h] = x @ Wq ---------------------------------
                  sc_sb = work.tile([128, MT, NH], f32)
                  for m2 in range(MT // 4):
                      sc_ps = scps.tile([128, 4, NH], f32, tag="sc")
                      for m4 in range(4):
                          m = m2 * 4 + m4
                          for k in range(KT):
                              nc.tensor.matmul(
                                  sc_ps[:, m4, :],
                                  xt_sb[:, m, k, :],
                                  wq_sb[:, k, :],
                                  start=(k == 0),
                                  stop=(k == KT - 1),
                              )
                      nc.vector.tensor_copy(sc_sb[:, m2 * 4:(m2 + 1) * 4, :], sc_ps[:])

                  # --- e = exp(scores) (bf16); mask is folded into x and the
                  # Z moving operand, so no explicit mask multiply is needed.
                  # Split into per-group ops so the pooling matmuls can trail
                  # the score stream instead of waiting for all 16 chunks -----
                  e_sb = work.tile([128, MT, NH], bf16)
                  for m2 in range(MT // 4):
                      nc.scalar.activation(
                          e_sb[:, m2 * 4:(m2 + 1) * 4, :],
                          sc_sb[:, m2 * 4:(m2 + 1) * 4, :],
                          mybir.ActivationFunctionType.Exp,
                          scale=escale_sb[:],
                      )

                  # --- Z[h] = sum_t e (output oriented [NH, 1]) --------------
                  z_ps = tps.tile([NH, 1], f32, tag="tps")
                  for m in range(MT):
                      nc.tensor.matmul(
                          z_ps[:],
                          e_sb[:, m, :],
                          mcol_sb[:, b, m:m + 1],
                          start=(m == 0),
                          stop=(m == MT - 1),
                      )
                  z_sb = work.tile([NH, 1], f32)
                  nc.vector.tensor_copy(z_sb[:], z_ps[:])
                  # broadcast 1/Z down all 128 partitions: [NH,1] -T-> [1,NH]
                  # -K=1 matmul-> [128,NH] -reciprocal-> sbuf
                  zt_ps = tps.tile([1, NH], f32, tag="tps")
                  nc.tensor.transpose(zt_ps[:], z_sb[:], idf_sb[0:NH, 0:NH])
                  zt_sb = work.tile([1, NH], f32)
                  nc.vector.tensor_copy(zt_sb[:], zt_ps[:])
                  zbc_ps = tps.tile([128, NH], f32, tag="tps")
                  nc.tensor.matmul(
                      zbc_ps[:], onescol_sb[:], zt_sb[:], start=True, stop=True
                  )
                  zinv_sb = work.tile([128, NH], f32)
                  nc.vector.reciprocal(zinv_sb[:], zbc_ps[:])

                  # --- px[h, c] = e.T @ x (unnormalized pool) ----------------
                  px_sb = work.tile([NH, HIDDEN], f32)
                  for c2 in range(2):
                      px_ps = pxps.tile([NH, 512], f32, tag="px")
                      for m in range(MT):
                          nc.tensor.matmul(
                              px_ps[:],
                              e_sb[:, m, :],
                              x_sb[:, m, c2 * 512:(c2 + 1) * 512],
                              start=(m == 0),
                              stop=(m == MT - 1),
                          )
                      nc.vector.tensor_copy(px_sb[:, c2 * 512:(c2 + 1) * 512], px_ps[:])

                  # --- pxT: [C-tile, h] with b packed in the free dim --------
                  for k in range(KT):
                      pxt_ps = tps.tile([128, NH], f32, tag="tps")
                      nc.tensor.transpose(
                          pxt_ps[:], px_sb[:, k * 128:(k + 1) * 128], idf_sb[0:NH, 0:NH]
                      )
                      nc.vector.tensor_mul(pxall_sb[:, k, :, b], pxt_ps[:], zinv_sb[:])

              # --- stage 3: out1_raw[b, hd] = px @ Wv -------------------------
              out1_ps = bigps.tile([NB, HIDDEN], f32)
              for h in range(NH):
                  for k in range(KT):
                      nc.tensor.matmul(
                          out1_ps[:, h * HD:(h + 1) * HD],
                          pxall_sb[:, k, h, :],
                          wv_sb[:, k, h, :],
                          start=(k == 0),
                          stop=(k == KT - 1),
                      )

              # --- out1T: [hd-tile, b] (out1 already normalized; per-k copies
              # so the copy/transpose/stage-4 chain trails stage 3 head-by-head
              # instead of waiting for the full [4,1024] psum) -----------------
              out1n_sb = small.tile([NB, HIDDEN], f32)
              o1t_sb = small.tile([128, KT, NB], bf16)
              for k in range(KT):
                  nc.vector.tensor_copy(
                      out1n_sb[:, k * 128:(k + 1) * 128],
                      out1_ps[:, k * 128:(k + 1) * 128],
                  )
                  o1t_ps = tps.tile([128, NB], f32, tag="tps")
                  nc.tensor.transpose(
                      o1t_ps[:], out1n_sb[:, k * 128:(k + 1) * 128], idf_sb[0:NB, 0:NB]
                  )
                  nc.vector.tensor_copy(o1t_sb[:, k, :], o1t_ps[:])

              # --- stage 4: out = out1 @ out_w + bias -------------------------
              of_sb = small.tile([NB, PROJ], f32)
              of_ps0 = scps.tile([NB, 512], f32, tag="sc")
              of_ps1 = scps.tile([NB, 512], f32, tag="sc")
              for k in range(KT):
                  for p2, of_ps in ((0, of_ps0), (1, of_ps1)):
                      nc.tensor.matmul(
                          of_ps[:],
                          o1t_sb[:, k, :],
                          wo_sb[:, k, p2, :],
                          start=(k == 0),
                          stop=(k == KT - 1),
                      )
              for p2, of_ps in ((0, of_ps0), (1, of_ps1)):
                  nc.vector.tensor_add(
                      of_sb[:, p2 * 512:(p2 + 1) * 512],
                      of_ps[:],
                      biasrep_sb[:, p2 * 512:(p2 + 1) * 512],
                  )
              nc.sync.dma_start(out_d[:], of_sb[:])

    nc.compile()
    return nc


def _get_nc():
    global _CACHED_NC
    if _CACHED_NC is None:
        _CACHED_NC = _build_nc()
    return _CACHED_NC


def _prep_inputs(hidden_states, mask, kv_w, kv_b, out_w, out_b, query):
    """Host-side sharding + weight preprocessing -> per-core input maps."""
    x = np.ascontiguousarray(hidden_states, dtype=np.float32)
    mask = np.asarray(mask)
    kv_w = np.asarray(kv_w, dtype=np.float32)
    kv_b = np.asarray(kv_b, dtype=np.float32)
    out_w = np.asarray(out_w, dtype=np.float32)
    out_b = np.asarray(out_b, dtype=np.float32)
    query = np.asarray(query, dtype=np.float32)

    scale = 1.0 / HD ** 0.5
    Wk = kv_w[:, :HIDDEN]
    Wv = kv_w[:, HIDDEN:]
    qh = query.reshape(NH, HD)
    # fold query into the k-projection: Wq[c, h]
    Wq = np.einsum("chd,hd->ch", Wk.reshape(HIDDEN, NH, HD), qh) * scale
    bias_final = kv_b[HIDDEN:] @ out_w + out_b  # v-bias is exact post-pool

    # dynamic power-of-2 fp8 scales (exactly unwound inside the exp activation)
    sw = 2.0 ** np.floor(np.log2(F8MAX / max(np.abs(Wq).max(), 1e-30)))
    sx = 2.0 ** np.floor(np.log2(F8MAX / max(np.abs(x).max(), 1e-30)))
    sx = min(sx, 1.0)
    escale = np.full((128, 1), 1.0 / (sw * sx), np.float32)
    wq_r = np.ascontiguousarray(
        (Wq * sw).reshape(KT, 128, NH).transpose(1, 0, 2)
    ).astype(F8)  # [128, KT, NH], fp8 with exp-unwound scale
    wv_r = np.ascontiguousarray(
        Wv.reshape(KT, 128, NH, HD).transpose(1, 0, 2, 3)
    ).astype(BF16)  # [128, KT, NH, HD]
    wo_r = np.ascontiguousarray(
        out_w.reshape(KT, 128, 2, 512).transpose(1, 0, 2, 3)
    ).astype(BF16)  # [128, KT, 2, 512]
    onescol = np.ones((1, 128), np.float32)
    idf = np.eye(128, dtype=np.float32)

    mvalid = (mask != 0).astype(np.float32)      # reference masks where mask == 0
    x_bf = (x * mvalid[:, :, None]).astype(BF16)  # pre-masked pooling copy [B, T, C]
    # xt chunked layout: xtr[b, p, m, k, t] = x[b, m*128+t, k*128+p]; per-partition
    # rows are contiguous in (m, k, t) so m-range DMA slices stay order-aligned
    xt_bf = np.ascontiguousarray(
        (x * sx).reshape(B, MT, 128, KT, 128).transpose(0, 4, 1, 3, 2)
    ).astype(F8)

    in_maps = []
    for c in range(NCORES):
        sl = slice(c * NB, (c + 1) * NB)
        # mcol[p, b, m] = valid(mask[b, m*128+p])
        mcol = np.ascontiguousarray(
            mvalid[sl].reshape(NB, MT, 128).transpose(2, 0, 1)
        ).astype(BF16)
        in_maps.append({
            "x": x_bf[sl],
            "xt": xt_bf[sl],
            "wq": wq_r,
            "wv": wv_r,
            "wo": wo_r,
            "mcol": mcol,
            "biasrep": np.ascontiguousarray(
                np.broadcast_to(bias_final[None, :], (NB, PROJ))
            ),
            "onescol": onescol,
            "idf": idf,
            "escale": escale,
        })
    return in_maps


def kernel(hidden_states, mask, kv_w, kv_b, out_w, out_b, query, **_unused):
    from concourse.bass_utils import run_bass_kernel_spmd

    nc = _get_nc()
    in_maps = _prep_inputs(hidden_states, mask, kv_w, kv_b, out_w, out_b, query)
    res = run_bass_kernel_spmd(nc, in_maps, list(range(NCORES)))
    out = np.concatenate([res.results[i]["out"] for i in range(NCORES)], axis=0)
    return out.astype(np.float32)



# revision 18
# speedup vs baseline: 1.8965x; 1.8965x over previous
"""AudioAttentionPooler Trainium2 kernel (v2).

Algorithm (algebraically identical to the reference, ~60x fewer FLOPs):
  scores[b,t,h] = x[b,t,:] @ Wq[:,h]        Wq = fold(query*scale, kv_w_k)  [C,h]
  (k-bias shifts scores uniformly along t -> softmax-invariant -> dropped)
  e = exp(scores)                           (mask folded into x and Z instead)
  Z[b,h] = sum_t e[b,t,h] * mask[b,t]
  px[b,h,:] = sum_t e[b,t,h] * (mask[b,t] * x[b,t,:])   (pool BEFORE v-proj)
  out1[b,h*64+d] = (px[b,h,:] @ Wv[:,h*64+d]) / Z[b,h]
  out = out1 @ out_w + (kv_b_v @ out_w + out_b)   (v-bias exact: attn sums to 1)

Sharding: data-parallel over batch, 4 batch elements per core x 8 cores.
x is fed in both [T,C] (bf16, pooling) and [C,T] (fp8, scores) layouts.

v2 structure changes vs v1:
  - out-DMA issued at loop HEAD (prev rep's result) so next rep's input
    loads never queue behind the compute tail; final DMA after the loop.
  - exp reads scores directly from PSUM (no DVE staging copy).
  - Z computed transposed ([1,16] via mcol-stationary matmul) - no
    transpose chain before the broadcast matmul.
  - stage 3 emits out1 TRANSPOSED (wv-stationary) straight into the
    [128, KT, NB] layout stage 4 needs: kills 8 PE transposes + 15 DVE
    copies in the tail.
  - weights stream at b==2 so stage 3/4 never wait on them.
  - per-partition-contiguous DRAM layouts, 1-2MB DMA chunks.
"""

import numpy as np
import ml_dtypes

BF16 = ml_dtypes.bfloat16

HIDDEN = 1024
NH = 16
HD = 64
PROJ = 1024
B, T = 32, 2048
NCORES = 8
NB = B // NCORES          # 4 batch elems per core
KT = HIDDEN // 128        # 8 C-tiles
MT = T // 128             # 16 T-chunks
F8 = ml_dtypes.float8_e4m3
F8MAX = 240.0             # conservative e4m3 range cap

_CACHED_NC = None


def _build_nc(reps=1, mode="full"):
    import concourse.bacc as bacc
    import concourse.mybir as mybir
    import concourse.tile as tile

    f32 = mybir.dt.float32
    bf16 = mybir.dt.bfloat16
    f8 = mybir.dt.float8e4

    dma = mode in ("full", "dma")
    compute = mode in ("full", "compute")

    nc = bacc.Bacc("TRN2", target_bir_lowering=False, debug=False)

    xp_d = nc.dram_tensor("xp", [NB, 128, MT, HIDDEN], bf16, kind="ExternalInput")
    xt_d = nc.dram_tensor("xt", [NB, 128, MT, KT, 128], f8, kind="ExternalInput")
    wq_d = nc.dram_tensor("wq", [128, KT, NH], f8, kind="ExternalInput")
    wv_d = nc.dram_tensor("wv", [128, NH // 2, KT, 2, HD], bf16, kind="ExternalInput")
    wo_d = nc.dram_tensor("wo", [128, KT, 2, 512], bf16, kind="ExternalInput")
    mcol_d = nc.dram_tensor("mcol", [128, NB, MT], bf16, kind="ExternalInput")
    biasrep_d = nc.dram_tensor("biasrep", [NB, PROJ], f32, kind="ExternalInput")
    idf_d = nc.dram_tensor("idf", [NH, NH], bf16, kind="ExternalInput")
    escale_d = nc.dram_tensor("escale", [128, 1], f32, kind="ExternalInput")
    out_d = nc.dram_tensor("out", [NB, PROJ], f32, kind="ExternalOutput")

    from contextlib import nullcontext

    with tile.TileContext(nc) as tc:
        with (
            tc.tile_pool(name="consts", bufs=1) as consts,
            tc.tile_pool(name="xppool", bufs=3) as xppool,
            tc.tile_pool(name="xtpool", bufs=2) as xtpool,
            tc.tile_pool(name="epool", bufs=2) as epool,
            tc.tile_pool(name="work", bufs=4) as work,
            tc.tile_pool(name="pxsb", bufs=2) as pxsb,
            tc.tile_pool(name="pxpool", bufs=2) as pxpool,
            tc.tile_pool(name="opool", bufs=1) as opool,
            tc.tile_pool(name="scps", bufs=1, space="PSUM") as scps,
            tc.tile_pool(name="pxps", bufs=1, space="PSUM") as pxps,
            tc.tile_pool(name="tps", bufs=2, space="PSUM") as tps,
            tc.tile_pool(name="o1ps", bufs=1, space="PSUM") as o1ps,
            tc.tile_pool(name="ofps", bufs=1, space="PSUM") as ofps,
        ):
            wq_sb = consts.tile([128, KT, NH], f8)
            wv_sb = consts.tile([128, NH // 2, KT, 2, HD], bf16)
            wo_sb = consts.tile([128, KT, 2, 512], bf16)
            mcol_sb = consts.tile([128, NB, MT], bf16)
            biasrep_sb = consts.tile([NB, PROJ], f32)
            idf_sb = consts.tile([NH, NH], bf16)
            escale_sb = consts.tile([128, 1], f32)

            # pre-loop: wq + small consts
            nc.sync.dma_start(wq_sb[:], wq_d[:])
            nc.sync.dma_start(escale_sb[:], escale_d[:])
            nc.sync.dma_start(mcol_sb[:], mcol_d[:])
            nc.sync.dma_start(idf_sb[:], idf_d[:])
            nc.sync.dma_start(biasrep_sb[:], biasrep_d[:])

            rep_ctx = tc.For_i(0, reps, 1) if reps > 1 else nullcontext()
            with rep_ctx:
              pxall_sb = pxpool.tile([128, KT, NH, NB], bf16)
              # stage-4 accumulator carries the bias as its PSUM start value
              # (loaded here, off the critical path; stage 4 uses start=False)
              of_ps = ofps.tile([NB, 2, 512], f32)
              nc.vector.tensor_copy(
                  of_ps[:], biasrep_sb[:].rearrange("b (p c) -> b p c", p=2)
              )
              # --- DMA program order = HWDGE ring service order: ------------
              # per b: xt (scores) then xp (pooling chases its chunks);
              # weights last so they stream during the compute tail
              for b in range(NB):
                  xt_sb = xtpool.tile([128, MT, KT, 128], f8)
                  xp_sb = xppool.tile([128, MT, HIDDEN], bf16)
                  if dma:
                      for h2 in range(2):
                          nc.sync.dma_start(
                              xt_sb[:, h2 * 8:(h2 + 1) * 8],
                              xt_d[b, :, h2 * 8:(h2 + 1) * 8],
                          )
                      for h2 in range(2):
                          nc.sync.dma_start(
                              xp_sb[:, h2 * 8:(h2 + 1) * 8],
                              xp_d[b, :, h2 * 8:(h2 + 1) * 8],
                          )
                  else:
                      nc.sync.dma_start(xt_sb[:, 0, 0, 0:4], xt_d[b, :, 0, 0, 0:4])
                      nc.sync.dma_start(xp_sb[:, 0, 0:4], xp_d[b, :, 0, 0:4])
                  if b == NB - 1:
                      if dma:
                          for j in range(NH // 2):
                              nc.sync.dma_start(wv_sb[:, j], wv_d[:, j])
                          for k in range(KT):
                              nc.sync.dma_start(wo_sb[:, k], wo_d[:, k])
                      else:
                          nc.sync.dma_start(wv_sb[:, 0, 0, 0, 0:4], wv_d[:, 0, 0, 0, 0:4])
                          nc.sync.dma_start(wo_sb[:, 0, 0, 0:4], wo_d[:, 0, 0, 0:4])

                  if not compute:
                      continue
                  # --- scores + exp: e[t,h] = exp(escale * x @ Wq) -----------
                  # exp reads the PSUM accumulator directly (ACT PSUM port is
                  # faster than its SBUF port; saves a DVE staging copy)
                  e_sb = epool.tile([128, MT, NH], bf16)
                  for m2 in range(MT // 4):
                      sc_ps = scps.tile([128, 4, NH], f32, tag="sc")
                      for m4 in range(4):
                          m = m2 * 4 + m4
                          for k in range(KT):
                              nc.tensor.matmul(
                                  sc_ps[:, m4, :],
                                  xt_sb[:, m, k, :],
                                  wq_sb[:, k, :],
                                  start=(k == 0),
                                  stop=(k == KT - 1),
                              )
                      nc.scalar.activation(
                          e_sb[:, m2 * 4:(m2 + 1) * 4, :],
                          sc_ps[:],
                          mybir.ActivationFunctionType.Exp,
                          scale=escale_sb[:],
                      )

                  # --- Z[h,1] = sum_t e*mask; 1/Z as per-partition scalar ----
                  z_ps = tps.tile([NH, 1], f32, tag="tps")
                  for m in range(MT):
                      nc.tensor.matmul(
                          z_ps[:],
                          e_sb[:, m, :],
                          mcol_sb[:, b, m:m + 1],
                          start=(m == 0),
                          stop=(m == MT - 1),
                      )
                  zinv_sb = work.tile([NH, 1], f32)
                  nc.vector.reciprocal(zinv_sb[:], z_ps[:])

                  # --- px[h, c] = e.T @ x; m-outer so pooling consumes each
                  # arriving xp chunk fully before the next is needed ---------
                  px_ps = pxps.tile([NH, 2, 512], f32, tag="px")
                  for m in range(MT):
                      for c2 in range(2):
                          nc.tensor.matmul(
                              px_ps[:, c2, :],
                              e_sb[:, m, :],
                              xp_sb[:, m, c2 * 512:(c2 + 1) * 512],
                              start=(m == 0),
                              stop=(m == MT - 1),
                          )
                  px_sb = pxsb.tile([NH, HIDDEN], bf16)
                  nc.vector.tensor_scalar_mul(
                      px_sb[:].rearrange("h (p c) -> h p c", p=2), px_ps[:],
                      zinv_sb[:],
                  )

                  # --- pxT: [C-tile, h] with b packed in the free dim --------
                  for k in range(KT):
                      pxt_ps = tps.tile([128, NH], bf16, tag="tps")
                      nc.tensor.transpose(
                          pxt_ps[:], px_sb[:, k * 128:(k + 1) * 128], idf_sb[:]
                      )
                      nc.vector.tensor_copy(pxall_sb[:, k, :, b], pxt_ps[:])

              if compute:
                  # --- stage 3 (transposed) interleaved with stage 4 ---------
                  # head h -> partition (h%2)*64, free group h//2: exactly the
                  # [128, KT, NB] layout stage 4 consumes. PSUM accumulation
                  # groups sharing a bank must not interleave, so each head
                  # pair j finishes its full k-contraction before the next;
                  # stage 4's j-th step then trails pair by pair.
                  o1t_ps = o1ps.tile([128, KT, NB], f32)
                  o1t_sb = opool.tile([128, KT, NB], bf16)
                  for j in range(KT):
                      for hh in range(2):
                          h = 2 * j + hh
                          lo = hh * 64
                          for k in range(KT):
                              nc.tensor.matmul(
                                  o1t_ps[lo:lo + 64, j, :],
                                  wv_sb[:, j, k, hh, :],
                                  pxall_sb[:, k, h, :],
                                  start=(k == 0),
                                  stop=(k == KT - 1),
                              )
                      nc.vector.tensor_copy(o1t_sb[:, j, :], o1t_ps[:, j, :])
                      for p2 in range(2):
                          nc.tensor.matmul(
                              of_ps[:, p2, :],
                              o1t_sb[:, j, :],
                              wo_sb[:, j, p2, :],
                              start=False,
                              stop=(j == KT - 1),
                          )
                  of_sb = opool.tile([NB, PROJ], f32)
                  nc.vector.tensor_copy(of_sb[:, 0:512], of_ps[:, 0, :])
                  nc.vector.tensor_copy(of_sb[:, 512:1024], of_ps[:, 1, :])
                  nc.sync.dma_start(out_d[:], of_sb[:])
              else:
                  nc.sync.dma_start(out_d[:], biasrep_sb[:])

    nc.compile()
    return nc


def _get_nc():
    global _CACHED_NC
    if _CACHED_NC is None:
        _CACHED_NC = _build_nc()
    return _CACHED_NC


def _prep_inputs(hidden_states, mask, kv_w, kv_b, out_w, out_b, query):
    """Host-side sharding + weight preprocessing -> per-core input maps."""
    x = np.ascontiguousarray(hidden_states, dtype=np.float32)
    mask = np.asarray(mask)
    kv_w = np.asarray(kv_w, dtype=np.float32)
    kv_b = np.asarray(kv_b, dtype=np.float32)
    out_w = np.asarray(out_w, dtype=np.float32)
    out_b = np.asarray(out_b, dtype=np.float32)
    query = np.asarray(query, dtype=np.float32)

    scale = 1.0 / HD ** 0.5
    Wk = kv_w[:, :HIDDEN]
    Wv = kv_w[:, HIDDEN:]
    qh = query.reshape(NH, HD)
    # fold query into the k-projection: Wq[c, h]
    Wq = np.einsum("chd,hd->ch", Wk.reshape(HIDDEN, NH, HD), qh) * scale
    bias_final = kv_b[HIDDEN:] @ out_w + out_b  # v-bias is exact post-pool

    # dynamic power-of-2 fp8 scales (exactly unwound inside the exp activation)
    sw = 2.0 ** np.floor(np.log2(F8MAX / max(np.abs(Wq).max(), 1e-30)))
    sx = 2.0 ** np.floor(np.log2(F8MAX / max(np.abs(x).max(), 1e-30)))
    sx = min(sx, 1.0)
    escale = np.full((128, 1), 1.0 / (sw * sx), np.float32)
    wq_r = np.ascontiguousarray(
        (Wq * sw).reshape(KT, 128, NH).transpose(1, 0, 2)
    ).astype(F8)  # [128, KT, NH], fp8 with exp-unwound scale
    wv_r = np.ascontiguousarray(
        Wv.reshape(KT, 128, NH // 2, 2, HD).transpose(1, 2, 0, 3, 4)
    ).astype(BF16)  # [128, pair, KT, 2, HD]
    wo_r = np.ascontiguousarray(
        out_w.reshape(KT, 128, 2, 512).transpose(1, 0, 2, 3)
    ).astype(BF16)  # [128, KT, 2, 512]
    idf = np.eye(NH, dtype=BF16)

    mvalid = (mask != 0).astype(np.float32)      # reference masks where mask == 0
    # xp[b, p, m, c] = (x*mask)[b, m*128+p, c]: per-partition contiguous 32KB
    xp_bf = np.ascontiguousarray(
        (x * mvalid[:, :, None]).reshape(B, MT, 128, HIDDEN).transpose(0, 2, 1, 3)
    ).astype(BF16)
    # xt[b, p, m, k, t] = x[b, m*128+t, k*128+p]: per-partition contiguous 16KB
    xt_bf = np.ascontiguousarray(
        (x * sx).reshape(B, MT, 128, KT, 128).transpose(0, 4, 1, 3, 2)
    ).astype(F8)

    in_maps = []
    for c in range(NCORES):
        sl = slice(c * NB, (c + 1) * NB)
        # mcol[p, b, m] = valid(mask[b, m*128+p])
        mcol = np.ascontiguousarray(
            mvalid[sl].reshape(NB, MT, 128).transpose(2, 0, 1)
        ).astype(BF16)
        in_maps.append({
            "xp": xp_bf[sl],
            "xt": xt_bf[sl],
            "wq": wq_r,
            "wv": wv_r,
            "wo": wo_r,
            "mcol": mcol,
            "biasrep": np.ascontiguousarray(
                np.broadcast_to(bias_final[None, :], (NB, PROJ))
            ),
            "idf": idf,
            "escale": escale,
        })
    return in_maps


def kernel(hidden_states, mask, kv_w, kv_b, out_w, out_b, query, **_unused):
    from concourse.bass_utils import run_bass_kernel_spmd

    nc = _get_nc()
    in_maps = _prep_inputs(hidden_states, mask, kv_w, kv_b, out_w, out_b, query)
    res = run_bass_kernel_spmd(nc, in_maps, list(range(NCORES)))
    out = np.concatenate([res.results[i]["out"] for i in range(NCORES)], axis=0)
    return out.astype(np.float32)


# revision 22
# speedup vs baseline: 1.9140x; 1.0093x over previous
"""AudioAttentionPooler Trainium2 kernel (v2).

Algorithm (algebraically identical to the reference, ~60x fewer FLOPs):
  scores[b,t,h] = x[b,t,:] @ Wq[:,h]        Wq = fold(query*scale, kv_w_k)  [C,h]
  (k-bias shifts scores uniformly along t -> softmax-invariant -> dropped)
  e = exp(scores)                           (mask folded into x and Z instead)
  Z[b,h] = sum_t e[b,t,h] * mask[b,t]
  px[b,h,:] = sum_t e[b,t,h] * (mask[b,t] * x[b,t,:])   (pool BEFORE v-proj)
  out1[b,h*64+d] = (px[b,h,:] @ Wv[:,h*64+d]) / Z[b,h]
  out = out1 @ out_w + (kv_b_v @ out_w + out_b)   (v-bias exact: attn sums to 1)

Sharding: data-parallel over batch, 4 batch elements per core x 8 cores.
x is fed in both [T,C] (bf16, pooling) and [C,T] (fp8, scores) layouts.

Key structure (DMA program order == HWDGE ring service order, so the
stream is scheduled so the last-arriving bytes feed the least compute):
  - per b: xt chunks (scores) then xp chunks (pooling m-outer chases
    them); wv (pair-major chunks) + wo stream LAST, during the compute
    tail, with stage 3/4 chasing their chunks.
  - exp reads scores directly from PSUM (ACT PSUM port; no DVE staging).
  - 1/Z kept as a [16,1] per-partition scalar, folded into the px
    psum->sbuf copy via tensor_scalar_mul (no broadcast matmul chain).
  - stage 3 emits out1 TRANSPOSED (wv-stationary) into the [128, KT, NB]
    layout stage 4 consumes; head pair j completes its full k-loop before
    the next (PSUM groups sharing a bank must not interleave), and
    stage 4's j-th step + bias-preloaded PSUM accumulator trail it.
  - per-partition-contiguous DRAM layouts, 1-2MB DMA chunks.
"""

import numpy as np
import ml_dtypes

BF16 = ml_dtypes.bfloat16

HIDDEN = 1024
NH = 16
HD = 64
PROJ = 1024
B, T = 32, 2048
NCORES = 8
NB = B // NCORES          # 4 batch elems per core
KT = HIDDEN // 128        # 8 C-tiles
MT = T // 128             # 16 T-chunks
F8 = ml_dtypes.float8_e4m3
F8MAX = 240.0             # conservative e4m3 range cap

_CACHED_NC = None


def _build_nc(reps=1, mode="full"):
    import concourse.bacc as bacc
    import concourse.mybir as mybir
    import concourse.tile as tile

    f32 = mybir.dt.float32
    bf16 = mybir.dt.bfloat16
    f8 = mybir.dt.float8e4

    dma = mode in ("full", "dma")
    compute = mode in ("full", "compute")

    nc = bacc.Bacc("TRN2", target_bir_lowering=False, debug=False)

    xp_d = nc.dram_tensor("xp", [NB, 128, MT, HIDDEN], bf16, kind="ExternalInput")
    xt_d = nc.dram_tensor("xt", [NB, 128, MT, KT, 128], f8, kind="ExternalInput")
    wq_d = nc.dram_tensor("wq", [128, KT, NH], f8, kind="ExternalInput")
    wv_d = nc.dram_tensor("wv", [128, NH // 2, KT, 2, HD], bf16, kind="ExternalInput")
    wo_d = nc.dram_tensor("wo", [128, KT, 2, 512], bf16, kind="ExternalInput")
    mcol_d = nc.dram_tensor("mcol", [128, NB, MT], bf16, kind="ExternalInput")
    biasrep_d = nc.dram_tensor("biasrep", [NB, PROJ], f32, kind="ExternalInput")
    idf_d = nc.dram_tensor("idf", [NH, NH], bf16, kind="ExternalInput")
    escale_d = nc.dram_tensor("escale", [128, 1], f32, kind="ExternalInput")
    out_d = nc.dram_tensor("out", [NB, PROJ], f32, kind="ExternalOutput")

    from contextlib import nullcontext

    with tile.TileContext(nc) as tc:
        with (
            tc.tile_pool(name="consts", bufs=1) as consts,
            tc.tile_pool(name="xppool", bufs=3) as xppool,
            tc.tile_pool(name="xtpool", bufs=2) as xtpool,
            tc.tile_pool(name="epool", bufs=2) as epool,
            tc.tile_pool(name="work", bufs=4) as work,
            tc.tile_pool(name="pxsb", bufs=2) as pxsb,
            tc.tile_pool(name="pxpool", bufs=2) as pxpool,
            tc.tile_pool(name="opool", bufs=1) as opool,
            tc.tile_pool(name="scps", bufs=1, space="PSUM") as scps,
            tc.tile_pool(name="pxps", bufs=1, space="PSUM") as pxps,
            tc.tile_pool(name="tps", bufs=2, space="PSUM") as tps,
            tc.tile_pool(name="o1ps", bufs=1, space="PSUM") as o1ps,
            tc.tile_pool(name="ofps", bufs=1, space="PSUM") as ofps,
        ):
            wq_sb = consts.tile([128, KT, NH], f8)
            wv_sb = consts.tile([128, NH // 2, KT, 2, HD], bf16)
            wo_sb = consts.tile([128, KT, 2, 512], bf16)
            mcol_sb = consts.tile([128, NB, MT], bf16)
            biasrep_sb = consts.tile([NB, PROJ], f32)
            idf_sb = consts.tile([NH, NH], bf16)
            escale_sb = consts.tile([128, 1], f32)

            # pre-loop: wq + small consts
            nc.sync.dma_start(wq_sb[:], wq_d[:])
            nc.sync.dma_start(escale_sb[:], escale_d[:])
            nc.sync.dma_start(mcol_sb[:], mcol_d[:])
            nc.sync.dma_start(idf_sb[:], idf_d[:])
            nc.sync.dma_start(biasrep_sb[:], biasrep_d[:])

            rep_ctx = tc.For_i(0, reps, 1) if reps > 1 else nullcontext()
            with rep_ctx:
              pxall_sb = pxpool.tile([128, KT, NH, NB], bf16)
              # stage-4 accumulator carries the bias as its PSUM start value
              # (loaded here, off the critical path; stage 4 uses start=False)
              of_ps = ofps.tile([NB, 2, 512], f32)
              nc.vector.tensor_copy(
                  of_ps[:], biasrep_sb[:].rearrange("b (p c) -> b p c", p=2)
              )
              # --- DMA program order = HWDGE ring service order: ------------
              # per b: xt (scores) then xp (pooling chases its chunks);
              # weights last so they stream during the compute tail
              for b in range(NB):
                  xt_sb = xtpool.tile([128, MT, KT, 128], f8)
                  xp_sb = xppool.tile([128, MT, HIDDEN], bf16)
                  if dma:
                      for h2 in range(2):
                          nc.sync.dma_start(
                              xt_sb[:, h2 * 8:(h2 + 1) * 8],
                              xt_d[b, :, h2 * 8:(h2 + 1) * 8],
                          )
                      for h2 in range(2):
                          nc.sync.dma_start(
                              xp_sb[:, h2 * 8:(h2 + 1) * 8],
                              xp_d[b, :, h2 * 8:(h2 + 1) * 8],
                          )
                  else:
                      nc.sync.dma_start(xt_sb[:, 0, 0, 0:4], xt_d[b, :, 0, 0, 0:4])
                      nc.sync.dma_start(xp_sb[:, 0, 0:4], xp_d[b, :, 0, 0:4])
                  if b == NB - 1:
                      if dma:
                          for j in range(NH // 2):
                              nc.sync.dma_start(wv_sb[:, j], wv_d[:, j])
                          for k in range(KT):
                              nc.sync.dma_start(wo_sb[:, k], wo_d[:, k])
                      else:
                          nc.sync.dma_start(wv_sb[:, 0, 0, 0, 0:4], wv_d[:, 0, 0, 0, 0:4])
                          nc.sync.dma_start(wo_sb[:, 0, 0, 0:4], wo_d[:, 0, 0, 0:4])

                  if not compute:
                      continue
                  # --- scores + exp: e[t,h] = exp(escale * x @ Wq) -----------
                  # exp reads the PSUM accumulator directly (ACT PSUM port is
                  # faster than its SBUF port; saves a DVE staging copy)
                  e_sb = epool.tile([128, MT, NH], bf16)
                  for m2 in range(MT // 4):
                      sc_ps = scps.tile([128, 4, NH], f32, tag="sc")
                      for m4 in range(4):
                          m = m2 * 4 + m4
                          for k in range(KT):
                              nc.tensor.matmul(
                                  sc_ps[:, m4, :],
                                  xt_sb[:, m, k, :],
                                  wq_sb[:, k, :],
                                  start=(k == 0),
                                  stop=(k == KT - 1),
                              )
                      nc.scalar.activation(
                          e_sb[:, m2 * 4:(m2 + 1) * 4, :],
                          sc_ps[:],
                          mybir.ActivationFunctionType.Exp,
                          scale=escale_sb[:],
                      )

                  # --- Z[h,1] = sum_t e*mask; 1/Z as per-partition scalar ----
                  z_ps = tps.tile([NH, 1], f32, tag="tps")
                  for m in range(MT):
                      nc.tensor.matmul(
                          z_ps[:],
                          e_sb[:, m, :],
                          mcol_sb[:, b, m:m + 1],
                          start=(m == 0),
                          stop=(m == MT - 1),
                      )
                  zinv_sb = work.tile([NH, 1], f32)
                  nc.vector.reciprocal(zinv_sb[:], z_ps[:])

                  # --- px[h, c] = e.T @ x; m-outer so pooling consumes each
                  # arriving xp chunk fully before the next is needed ---------
                  px_ps = pxps.tile([NH, 2, 512], f32, tag="px")
                  for m in range(MT):
                      for c2 in range(2):
                          nc.tensor.matmul(
                              px_ps[:, c2, :],
                              e_sb[:, m, :],
                              xp_sb[:, m, c2 * 512:(c2 + 1) * 512],
                              start=(m == 0),
                              stop=(m == MT - 1),
                          )
                  px_sb = pxsb.tile([NH, HIDDEN], bf16)
                  nc.vector.tensor_scalar_mul(
                      px_sb[:].rearrange("h (p c) -> h p c", p=2), px_ps[:],
                      zinv_sb[:],
                  )

                  # --- pxT: [C-tile, h] with b packed in the free dim --------
                  for k in range(KT):
                      pxt_ps = tps.tile([128, NH], bf16, tag="tps")
                      nc.tensor.transpose(
                          pxt_ps[:], px_sb[:, k * 128:(k + 1) * 128], idf_sb[:]
                      )
                      nc.vector.tensor_copy(pxall_sb[:, k, :, b], pxt_ps[:])

              if compute:
                  # --- stage 3 (transposed) interleaved with stage 4 ---------
                  # head h -> partition (h%2)*64, free group h//2: exactly the
                  # [128, KT, NB] layout stage 4 consumes. PSUM accumulation
                  # groups sharing a bank must not interleave, so each head
                  # pair j finishes its full k-contraction before the next;
                  # stage 4's j-th step then trails pair by pair.
                  o1t_ps = o1ps.tile([128, KT, NB], f32)
                  o1t_sb = opool.tile([128, KT, NB], bf16)
                  for j in range(KT):
                      for hh in range(2):
                          h = 2 * j + hh
                          lo = hh * 64
                          for k in range(KT):
                              nc.tensor.matmul(
                                  o1t_ps[lo:lo + 64, j, :],
                                  wv_sb[:, j, k, hh, :],
                                  pxall_sb[:, k, h, :],
                                  start=(k == 0),
                                  stop=(k == KT - 1),
                              )
                      nc.vector.tensor_copy(o1t_sb[:, j, :], o1t_ps[:, j, :])
                      for p2 in range(2):
                          nc.tensor.matmul(
                              of_ps[:, p2, :],
                              o1t_sb[:, j, :],
                              wo_sb[:, j, p2, :],
                              start=False,
                              stop=(j == KT - 1),
                          )
                  of_sb = opool.tile([NB, PROJ], f32)
                  nc.vector.tensor_copy(of_sb[:, 0:512], of_ps[:, 0, :])
                  nc.vector.tensor_copy(of_sb[:, 512:1024], of_ps[:, 1, :])
                  nc.sync.dma_start(out_d[:], of_sb[:])
              else:
                  nc.sync.dma_start(out_d[:], biasrep_sb[:])

    nc.compile()
    return nc


def _get_nc():
    global _CACHED_NC
    if _CACHED_NC is None:
        _CACHED_NC = _build_nc()
    return _CACHED_NC


def _prep_inputs(hidden_states, mask, kv_w, kv_b, out_w, out_b, query):
    """Host-side sharding + weight preprocessing -> per-core input maps."""
    x = np.ascontiguousarray(hidden_states, dtype=np.float32)
    mask = np.asarray(mask)
    kv_w = np.asarray(kv_w, dtype=np.float32)
    kv_b = np.asarray(kv_b, dtype=np.float32)
    out_w = np.asarray(out_w, dtype=np.float32)
    out_b = np.asarray(out_b, dtype=np.float32)
    query = np.asarray(query, dtype=np.float32)

    scale = 1.0 / HD ** 0.5
    Wk = kv_w[:, :HIDDEN]
    Wv = kv_w[:, HIDDEN:]
    qh = query.reshape(NH, HD)
    # fold query into the k-projection: Wq[c, h]
    Wq = np.einsum("chd,hd->ch", Wk.reshape(HIDDEN, NH, HD), qh) * scale
    bias_final = kv_b[HIDDEN:] @ out_w + out_b  # v-bias is exact post-pool

    # dynamic power-of-2 fp8 scales (exactly unwound inside the exp activation)
    sw = 2.0 ** np.floor(np.log2(F8MAX / max(np.abs(Wq).max(), 1e-30)))
    sx = 2.0 ** np.floor(np.log2(F8MAX / max(np.abs(x).max(), 1e-30)))
    sx = min(sx, 1.0)
    escale = np.full((128, 1), 1.0 / (sw * sx), np.float32)
    wq_r = np.ascontiguousarray(
        (Wq * sw).reshape(KT, 128, NH).transpose(1, 0, 2)
    ).astype(F8)  # [128, KT, NH], fp8 with exp-unwound scale
    wv_r = np.ascontiguousarray(
        Wv.reshape(KT, 128, NH // 2, 2, HD).transpose(1, 2, 0, 3, 4)
    ).astype(BF16)  # [128, pair, KT, 2, HD]
    wo_r = np.ascontiguousarray(
        out_w.reshape(KT, 128, 2, 512).transpose(1, 0, 2, 3)
    ).astype(BF16)  # [128, KT, 2, 512]
    idf = np.eye(NH, dtype=BF16)

    mvalid = (mask != 0).astype(np.float32)      # reference masks where mask == 0
    # xp[b, p, m, c] = (x*mask)[b, m*128+p, c]: per-partition contiguous 32KB
    xp_bf = np.ascontiguousarray(
        (x * mvalid[:, :, None]).reshape(B, MT, 128, HIDDEN).transpose(0, 2, 1, 3)
    ).astype(BF16)
    # xt[b, p, m, k, t] = x[b, m*128+t, k*128+p]: per-partition contiguous 16KB
    xt_bf = np.ascontiguousarray(
        (x * sx).reshape(B, MT, 128, KT, 128).transpose(0, 4, 1, 3, 2)
    ).astype(F8)

    in_maps = []
    for c in range(NCORES):
        sl = slice(c * NB, (c + 1) * NB)
        # mcol[p, b, m] = valid(mask[b, m*128+p])
        mcol = np.ascontiguousarray(
            mvalid[sl].reshape(NB, MT, 128).transpose(2, 0, 1)
        ).astype(BF16)
        in_maps.append({
            "xp": xp_bf[sl],
            "xt": xt_bf[sl],
            "wq": wq_r,
            "wv": wv_r,
            "wo": wo_r,
            "mcol": mcol,
            "biasrep": np.ascontiguousarray(
                np.broadcast_to(bias_final[None, :], (NB, PROJ))
            ),
            "idf": idf,
            "escale": escale,
        })
    return in_maps


def kernel(hidden_states, mask, kv_w, kv_b, out_w, out_b, query, **_unused):
    from concourse.bass_utils import run_bass_kernel_spmd

    nc = _get_nc()
    in_maps = _prep_inputs(hidden_states, mask, kv_w, kv_b, out_w, out_b, query)
    res = run_bass_kernel_spmd(nc, in_maps, list(range(NCORES)))
    out = np.concatenate([res.results[i]["out"] for i in range(NCORES)], axis=0)
    return out.astype(np.float32)


# revision 25
# speedup vs baseline: 1.9569x; 1.0224x over previous
"""AudioAttentionPooler Trainium2 kernel (v2).

Algorithm (algebraically identical to the reference, ~60x fewer FLOPs):
  scores[b,t,h] = x[b,t,:] @ Wq[:,h]        Wq = fold(query*scale, kv_w_k)  [C,h]
  (k-bias shifts scores uniformly along t -> softmax-invariant -> dropped)
  e = exp(scores)                           (mask folded into x and Z instead)
  Z[b,h] = sum_t e[b,t,h] * mask[b,t]
  px[b,h,:] = sum_t e[b,t,h] * (mask[b,t] * x[b,t,:])   (pool BEFORE v-proj)
  out1[b,h*64+d] = (px[b,h,:] @ Wv[:,h*64+d]) / Z[b,h]
  out = out1 @ out_w + (kv_b_v @ out_w + out_b)   (v-bias exact: attn sums to 1)

Sharding: data-parallel over batch, 4 batch elements per core x 8 cores.
x is fed in both [T,C] (bf16, pooling) and [C,T] (fp8, scores) layouts.

Key structure (DMA program order == HWDGE ring service order, so the
stream is scheduled so the last-arriving bytes feed the least compute):
  - per b: xt chunks (scores) then xp chunks (pooling m-outer chases
    them); wv (pair-major chunks) + wo stream LAST, during the compute
    tail, with stage 3/4 chasing their chunks.
  - exp reads scores directly from PSUM (ACT PSUM port; no DVE staging).
  - 1/Z kept as a [16,1] per-partition scalar, folded into the px
    psum->sbuf copy via tensor_scalar_mul (no broadcast matmul chain).
  - stage 3 emits out1 TRANSPOSED (wv-stationary) into the [128, KT, NB]
    layout stage 4 consumes; head pair j completes its full k-loop before
    the next (PSUM groups sharing a bank must not interleave), and
    stage 4's j-th step + bias-preloaded PSUM accumulator trail it.
  - per-partition-contiguous DRAM layouts, 1-2MB DMA chunks.
"""

import numpy as np
import ml_dtypes

BF16 = ml_dtypes.bfloat16

HIDDEN = 1024
NH = 16
HD = 64
PROJ = 1024
B, T = 32, 2048
NCORES = 8
NB = B // NCORES          # 4 batch elems per core
KT = HIDDEN // 128        # 8 C-tiles
MT = T // 128             # 16 T-chunks
F8 = ml_dtypes.float8_e4m3
F8MAX = 240.0             # conservative e4m3 range cap

_CACHED_NC = None


def _build_nc(reps=1, mode="full"):
    import concourse.bacc as bacc
    import concourse.mybir as mybir
    import concourse.tile as tile

    f32 = mybir.dt.float32
    bf16 = mybir.dt.bfloat16
    f8 = mybir.dt.float8e4

    dma = mode in ("full", "dma")
    compute = mode in ("full", "compute")

    nc = bacc.Bacc("TRN2", target_bir_lowering=False, debug=False)

    xp_d = nc.dram_tensor("xp", [NB, 128, MT, HIDDEN], bf16, kind="ExternalInput")
    xt_d = nc.dram_tensor("xt", [NB, 128, MT, KT, 128], f8, kind="ExternalInput")
    wq_d = nc.dram_tensor("wq", [128, KT, NH], f8, kind="ExternalInput")
    wv_d = nc.dram_tensor("wv", [128, NH // 2, KT, 2, HD], bf16, kind="ExternalInput")
    wo_d = nc.dram_tensor("wo", [128, KT, 2, 512], bf16, kind="ExternalInput")
    mcol_d = nc.dram_tensor("mcol", [128, NB, MT], bf16, kind="ExternalInput")
    biasrep_d = nc.dram_tensor("biasrep", [NB, PROJ], f32, kind="ExternalInput")
    idf_d = nc.dram_tensor("idf", [NH, NH], bf16, kind="ExternalInput")
    escale_d = nc.dram_tensor("escale", [128, 1], f32, kind="ExternalInput")
    out_d = nc.dram_tensor("out", [NB, PROJ], f32, kind="ExternalOutput")

    from contextlib import nullcontext

    with tile.TileContext(nc) as tc:
        with (
            tc.tile_pool(name="consts", bufs=1) as consts,
            tc.tile_pool(name="xppool", bufs=3) as xppool,
            tc.tile_pool(name="xtpool", bufs=2) as xtpool,
            tc.tile_pool(name="epool", bufs=2) as epool,
            tc.tile_pool(name="work", bufs=4) as work,
            tc.tile_pool(name="pxsb", bufs=2) as pxsb,
            tc.tile_pool(name="pxpool", bufs=2) as pxpool,
            tc.tile_pool(name="opool", bufs=1) as opool,
            tc.tile_pool(name="scps", bufs=1, space="PSUM") as scps,
            tc.tile_pool(name="pxps", bufs=1, space="PSUM") as pxps,
            tc.tile_pool(name="tps", bufs=2, space="PSUM") as tps,
            tc.tile_pool(name="o1ps", bufs=1, space="PSUM") as o1ps,
            tc.tile_pool(name="ofps", bufs=1, space="PSUM") as ofps,
        ):
            wq_sb = consts.tile([128, KT, NH], f8)
            wv_sb = consts.tile([128, NH // 2, KT, 2, HD], bf16)
            wo_sb = consts.tile([128, KT, 2, 512], bf16)
            mcol_sb = consts.tile([128, NB, MT], bf16)
            biasrep_sb = consts.tile([NB, PROJ], f32)
            idf_sb = consts.tile([NH, NH], bf16)
            escale_sb = consts.tile([128, 1], f32)

            # pre-loop: wq + small consts
            nc.sync.dma_start(wq_sb[:], wq_d[:])
            nc.sync.dma_start(escale_sb[:], escale_d[:])
            nc.sync.dma_start(mcol_sb[:], mcol_d[:])
            nc.sync.dma_start(idf_sb[:], idf_d[:])
            nc.sync.dma_start(biasrep_sb[:], biasrep_d[:])

            rep_ctx = tc.For_i(0, reps, 1) if reps > 1 else nullcontext()
            with rep_ctx:
              pxall_sb = pxpool.tile([128, KT, NH, NB], bf16)
              # stage-4 accumulator carries the bias as its PSUM start value
              # (loaded here, off the critical path; stage 4 uses start=False)
              of_ps = ofps.tile([NB, 2, 512], f32)
              nc.vector.tensor_copy(
                  of_ps[:], biasrep_sb[:].rearrange("b (p c) -> b p c", p=2)
              )
              # --- DMA program order = HWDGE ring service order: ------------
              # per b: xt (scores) then xp (pooling chases its chunks);
              # weights last so they stream during the compute tail
              for b in range(NB):
                  xt_sb = xtpool.tile([128, MT, KT, 128], f8)
                  xp_sb = xppool.tile([128, MT, HIDDEN], bf16)
                  if dma:
                      for h2 in range(2):
                          nc.sync.dma_start(
                              xt_sb[:, h2 * 8:(h2 + 1) * 8],
                              xt_d[b, :, h2 * 8:(h2 + 1) * 8],
                          )
                      for h2 in range(2):
                          nc.sync.dma_start(
                              xp_sb[:, h2 * 8:(h2 + 1) * 8],
                              xp_d[b, :, h2 * 8:(h2 + 1) * 8],
                          )
                  else:
                      nc.sync.dma_start(xt_sb[:, 0, 0, 0:4], xt_d[b, :, 0, 0, 0:4])
                      nc.sync.dma_start(xp_sb[:, 0, 0:4], xp_d[b, :, 0, 0:4])
                  if b == NB - 1:
                      if dma:
                          for j in range(NH // 4):
                              nc.sync.dma_start(
                                  wv_sb[:, 2 * j:2 * j + 2], wv_d[:, 2 * j:2 * j + 2]
                              )
                          for i in range(KT // 2):
                              nc.sync.dma_start(
                                  wo_sb[:, 2 * i:2 * i + 2], wo_d[:, 2 * i:2 * i + 2]
                              )
                      else:
                          nc.sync.dma_start(wv_sb[:, 0, 0, 0, 0:4], wv_d[:, 0, 0, 0, 0:4])
                          nc.sync.dma_start(wo_sb[:, 0, 0, 0:4], wo_d[:, 0, 0, 0:4])

                  if not compute:
                      continue
                  # --- scores + exp: e[t,h] = exp(escale * x @ Wq) -----------
                  # exp reads the PSUM accumulator directly (ACT PSUM port is
                  # faster than its SBUF port; saves a DVE staging copy)
                  e_sb = epool.tile([128, MT, NH], bf16)
                  for m2 in range(MT // 4):
                      sc_ps = scps.tile([128, 4, NH], f32, tag="sc")
                      for m4 in range(4):
                          m = m2 * 4 + m4
                          for k in range(KT):
                              nc.tensor.matmul(
                                  sc_ps[:, m4, :],
                                  xt_sb[:, m, k, :],
                                  wq_sb[:, k, :],
                                  start=(k == 0),
                                  stop=(k == KT - 1),
                              )
                      nc.scalar.activation(
                          e_sb[:, m2 * 4:(m2 + 1) * 4, :],
                          sc_ps[:],
                          mybir.ActivationFunctionType.Exp,
                          scale=escale_sb[:],
                      )

                  # --- Z[h,1] = sum_t e*mask; 1/Z as per-partition scalar ----
                  z_ps = tps.tile([NH, 1], f32, tag="tps")
                  for m in range(MT):
                      nc.tensor.matmul(
                          z_ps[:],
                          e_sb[:, m, :],
                          mcol_sb[:, b, m:m + 1],
                          start=(m == 0),
                          stop=(m == MT - 1),
                      )
                  zinv_sb = work.tile([NH, 1], f32)
                  nc.vector.reciprocal(zinv_sb[:], z_ps[:])

                  # --- px[h, c] = e.T @ x; m-outer so pooling consumes each
                  # arriving xp chunk fully before the next is needed ---------
                  px_ps = pxps.tile([NH, 2, 512], f32, tag="px")
                  for m in range(MT):
                      for c2 in range(2):
                          nc.tensor.matmul(
                              px_ps[:, c2, :],
                              e_sb[:, m, :],
                              xp_sb[:, m, c2 * 512:(c2 + 1) * 512],
                              start=(m == 0),
                              stop=(m == MT - 1),
                          )
                  px_sb = pxsb.tile([NH, HIDDEN], bf16)
                  nc.vector.tensor_scalar_mul(
                      px_sb[:].rearrange("h (p c) -> h p c", p=2), px_ps[:],
                      zinv_sb[:],
                  )

                  # --- pxT: [C-tile, h] with b packed in the free dim --------
                  for k in range(KT):
                      pxt_ps = tps.tile([128, NH], bf16, tag="tps")
                      nc.tensor.transpose(
                          pxt_ps[:], px_sb[:, k * 128:(k + 1) * 128], idf_sb[:]
                      )
                      nc.vector.tensor_copy(pxall_sb[:, k, :, b], pxt_ps[:])

              if compute:
                  # --- stage 3 (transposed) interleaved with stage 4 ---------
                  # head h -> partition (h%2)*64, free group h//2: exactly the
                  # [128, KT, NB] layout stage 4 consumes. PSUM accumulation
                  # groups sharing a bank must not interleave, so each head
                  # pair j finishes its full k-contraction before the next;
                  # stage 4's j-th step then trails pair by pair.
                  o1t_ps = o1ps.tile([128, KT, NB], f32)
                  o1t_sb = opool.tile([128, KT, NB], bf16)
                  for j in range(KT):
                      for hh in range(2):
                          h = 2 * j + hh
                          lo = hh * 64
                          for k in range(KT):
                              nc.tensor.matmul(
                                  o1t_ps[lo:lo + 64, j, :],
                                  wv_sb[:, j, k, hh, :],
                                  pxall_sb[:, k, h, :],
                                  start=(k == 0),
                                  stop=(k == KT - 1),
                              )
                      nc.vector.tensor_copy(o1t_sb[:, j, :], o1t_ps[:, j, :])
                      for p2 in range(2):
                          nc.tensor.matmul(
                              of_ps[:, p2, :],
                              o1t_sb[:, j, :],
                              wo_sb[:, j, p2, :],
                              start=False,
                              stop=(j == KT - 1),
                          )
                  of_sb = opool.tile([NB, PROJ], f32)
                  nc.vector.tensor_copy(of_sb[:, 0:512], of_ps[:, 0, :])
                  nc.vector.tensor_copy(of_sb[:, 512:1024], of_ps[:, 1, :])
                  nc.sync.dma_start(out_d[:], of_sb[:])
              else:
                  nc.sync.dma_start(out_d[:], biasrep_sb[:])

    nc.compile()
    return nc


def _get_nc():
    global _CACHED_NC
    if _CACHED_NC is None:
        _CACHED_NC = _build_nc()
    return _CACHED_NC


def _prep_inputs(hidden_states, mask, kv_w, kv_b, out_w, out_b, query):
    """Host-side sharding + weight preprocessing -> per-core input maps."""
    x = np.ascontiguousarray(hidden_states, dtype=np.float32)
    mask = np.asarray(mask)
    kv_w = np.asarray(kv_w, dtype=np.float32)
    kv_b = np.asarray(kv_b, dtype=np.float32)
    out_w = np.asarray(out_w, dtype=np.float32)
    out_b = np.asarray(out_b, dtype=np.float32)
    query = np.asarray(query, dtype=np.float32)

    scale = 1.0 / HD ** 0.5
    Wk = kv_w[:, :HIDDEN]
    Wv = kv_w[:, HIDDEN:]
    qh = query.reshape(NH, HD)
    # fold query into the k-projection: Wq[c, h]
    Wq = np.einsum("chd,hd->ch", Wk.reshape(HIDDEN, NH, HD), qh) * scale
    bias_final = kv_b[HIDDEN:] @ out_w + out_b  # v-bias is exact post-pool

    # dynamic power-of-2 fp8 scales (exactly unwound inside the exp activation)
    sw = 2.0 ** np.floor(np.log2(F8MAX / max(np.abs(Wq).max(), 1e-30)))
    sx = 2.0 ** np.floor(np.log2(F8MAX / max(np.abs(x).max(), 1e-30)))
    sx = min(sx, 1.0)
    escale = np.full((128, 1), 1.0 / (sw * sx), np.float32)
    wq_r = np.ascontiguousarray(
        (Wq * sw).reshape(KT, 128, NH).transpose(1, 0, 2)
    ).astype(F8)  # [128, KT, NH], fp8 with exp-unwound scale
    wv_r = np.ascontiguousarray(
        Wv.reshape(KT, 128, NH // 2, 2, HD).transpose(1, 2, 0, 3, 4)
    ).astype(BF16)  # [128, pair, KT, 2, HD]
    wo_r = np.ascontiguousarray(
        out_w.reshape(KT, 128, 2, 512).transpose(1, 0, 2, 3)
    ).astype(BF16)  # [128, KT, 2, 512]
    idf = np.eye(NH, dtype=BF16)

    mvalid = (mask != 0).astype(np.float32)      # reference masks where mask == 0
    # xp[b, p, m, c] = (x*mask)[b, m*128+p, c]: per-partition contiguous 32KB
    xp_bf = np.ascontiguousarray(
        (x * mvalid[:, :, None]).reshape(B, MT, 128, HIDDEN).transpose(0, 2, 1, 3)
    ).astype(BF16)
    # xt[b, p, m, k, t] = x[b, m*128+t, k*128+p]: per-partition contiguous 16KB
    xt_bf = np.ascontiguousarray(
        (x * sx).reshape(B, MT, 128, KT, 128).transpose(0, 4, 1, 3, 2)
    ).astype(F8)

    in_maps = []
    for c in range(NCORES):
        sl = slice(c * NB, (c + 1) * NB)
        # mcol[p, b, m] = valid(mask[b, m*128+p])
        mcol = np.ascontiguousarray(
            mvalid[sl].reshape(NB, MT, 128).transpose(2, 0, 1)
        ).astype(BF16)
        in_maps.append({
            "xp": xp_bf[sl],
            "xt": xt_bf[sl],
            "wq": wq_r,
            "wv": wv_r,
            "wo": wo_r,
            "mcol": mcol,
            "biasrep": np.ascontiguousarray(
                np.broadcast_to(bias_final[None, :], (NB, PROJ))
            ),
            "idf": idf,
            "escale": escale,
        })
    return in_maps


def kernel(hidden_states, mask, kv_w, kv_b, out_w, out_b, query, **_unused):
    from concourse.bass_utils import run_bass_kernel_spmd

    nc = _get_nc()
    in_maps = _prep_inputs(hidden_states, mask, kv_w, kv_b, out_w, out_b, query)
    res = run_bass_kernel_spmd(nc, in_maps, list(range(NCORES)))
    out = np.concatenate([res.results[i]["out"] for i in range(NCORES)], axis=0)
    return out.astype(np.float32)


# revision 30
# speedup vs baseline: 1.9993x; 1.0217x over previous
"""AudioAttentionPooler Trainium2 kernel (v2).

Algorithm (algebraically identical to the reference, ~60x fewer FLOPs):
  scores[b,t,h] = x[b,t,:] @ Wq[:,h]        Wq = fold(query*scale, kv_w_k)  [C,h]
  (k-bias shifts scores uniformly along t -> softmax-invariant -> dropped)
  e = exp(scores)                           (mask folded into x and Z instead)
  Z[b,h] = sum_t e[b,t,h] * mask[b,t]
  px[b,h,:] = sum_t e[b,t,h] * (mask[b,t] * x[b,t,:])   (pool BEFORE v-proj)
  out1[b,h*64+d] = (px[b,h,:] @ Wv[:,h*64+d]) / Z[b,h]
  out = out1 @ out_w + (kv_b_v @ out_w + out_b)   (v-bias exact: attn sums to 1)

Sharding: data-parallel over batch, 4 batch elements per core x 8 cores.
x is fed in both [T,C] (bf16, pooling) and [C,T] (fp8, scores) layouts.

Key structure (DMA program order == HWDGE ring service order, so the
stream is scheduled so the last-arriving bytes feed the least compute):
  - per b: xt chunks (scores) then xp chunks (pooling m-outer chases
    them); wv (pair-major chunks) + wo stream LAST, during the compute
    tail, with stage 3/4 chasing their chunks.
  - exp reads scores directly from PSUM (ACT PSUM port; no DVE staging).
  - 1/Z kept as a [16,1] per-partition scalar, folded into the px
    psum->sbuf copy via tensor_scalar_mul (no broadcast matmul chain).
  - stage 3 emits out1 TRANSPOSED (wv-stationary) into the [128, KT, NB]
    layout stage 4 consumes; head pair j completes its full k-loop before
    the next (PSUM groups sharing a bank must not interleave), and
    stage 4's j-th step + bias-preloaded PSUM accumulator trail it.
  - per-partition-contiguous DRAM layouts, 1-2MB DMA chunks.
"""

import numpy as np
import ml_dtypes

BF16 = ml_dtypes.bfloat16

HIDDEN = 1024
NH = 16
HD = 64
PROJ = 1024
B, T = 32, 2048
NCORES = 8
NB = B // NCORES          # 4 batch elems per core
KT = HIDDEN // 128        # 8 C-tiles
MT = T // 128             # 16 T-chunks
F8 = ml_dtypes.float8_e4m3
F8MAX = 240.0             # conservative e4m3 range cap

_CACHED_NC = None


def _build_nc(reps=1, mode="full"):
    import concourse.bacc as bacc
    import concourse.mybir as mybir
    import concourse.tile as tile

    f32 = mybir.dt.float32
    bf16 = mybir.dt.bfloat16
    f8 = mybir.dt.float8e4

    dma = mode in ("full", "dma")
    compute = mode in ("full", "compute")

    nc = bacc.Bacc("TRN2", target_bir_lowering=False, debug=False)

    xp_d = nc.dram_tensor("xp", [NB, 128, MT, HIDDEN], bf16, kind="ExternalInput")
    xt_d = nc.dram_tensor("xt", [NB, 128, MT, KT, 128], f8, kind="ExternalInput")
    wq_d = nc.dram_tensor("wq", [128, KT, NH], f8, kind="ExternalInput")
    wv_d = nc.dram_tensor("wv", [128, NH // 2, KT, 2, HD], bf16, kind="ExternalInput")
    wo_d = nc.dram_tensor("wo", [128, KT, 2, 512], bf16, kind="ExternalInput")
    mcol_d = nc.dram_tensor("mcol", [128, NB, MT], bf16, kind="ExternalInput")
    biasrep_d = nc.dram_tensor("biasrep", [NB, PROJ], f32, kind="ExternalInput")
    idf_d = nc.dram_tensor("idf", [NH, NH], bf16, kind="ExternalInput")
    escale_d = nc.dram_tensor("escale", [128, 1], f32, kind="ExternalInput")
    out_d = nc.dram_tensor("out", [NB, PROJ], f32, kind="ExternalOutput")

    from contextlib import nullcontext

    with tile.TileContext(nc) as tc:
        with (
            tc.tile_pool(name="consts", bufs=1) as consts,
            tc.tile_pool(name="xppool", bufs=3) as xppool,
            tc.tile_pool(name="xtpool", bufs=2) as xtpool,
            tc.tile_pool(name="epool", bufs=2) as epool,
            tc.tile_pool(name="work", bufs=4) as work,
            tc.tile_pool(name="pxsb", bufs=2) as pxsb,
            tc.tile_pool(name="pxpool", bufs=2) as pxpool,
            tc.tile_pool(name="opool", bufs=1) as opool,
            tc.tile_pool(name="scps", bufs=1, space="PSUM") as scps,
            tc.tile_pool(name="pxps", bufs=1, space="PSUM") as pxps,
            tc.tile_pool(name="tps", bufs=2, space="PSUM") as tps,
            tc.tile_pool(name="o1ps", bufs=1, space="PSUM") as o1ps,
            tc.tile_pool(name="ofps", bufs=1, space="PSUM") as ofps,
        ):
            wq_sb = consts.tile([128, KT, NH], f8)
            wv_sb = consts.tile([128, NH // 2, KT, 2, HD], bf16)
            wo_sb = consts.tile([128, KT, 2, 512], bf16)
            mcol_sb = consts.tile([128, NB, MT], bf16)
            biasrep_sb = consts.tile([NB, PROJ], f32)
            idf_sb = consts.tile([NH, NH], bf16)
            escale_sb = consts.tile([128, 1], f32)

            # pre-loop: wq + small consts
            nc.sync.dma_start(wq_sb[:], wq_d[:])
            nc.sync.dma_start(escale_sb[:], escale_d[:])
            nc.sync.dma_start(mcol_sb[:], mcol_d[:])
            nc.sync.dma_start(idf_sb[:], idf_d[:])
            nc.sync.dma_start(biasrep_sb[:], biasrep_d[:])

            rep_ctx = tc.For_i(0, reps, 1) if reps > 1 else nullcontext()
            with rep_ctx:
              pxall_sb = pxpool.tile([128, KT, NH, NB], bf16)
              # stage-4 accumulator carries the bias as its PSUM start value
              # (loaded here, off the critical path; stage 4 uses start=False)
              of_ps = ofps.tile([NB, 2, 512], f32)
              nc.vector.tensor_copy(
                  of_ps[:], biasrep_sb[:].rearrange("b (p c) -> b p c", p=2)
              )
              # --- DMA program order = HWDGE ring service order: ------------
              # per b: xt (scores) then xp (pooling chases its chunks);
              # weights last so they stream during the compute tail
              for b in range(NB):
                  xt_sb = xtpool.tile([128, MT, KT, 128], f8)
                  xp_sb = xppool.tile([128, MT, HIDDEN], bf16)
                  if dma:
                      for h2 in range(2):
                          nc.sync.dma_start(
                              xt_sb[:, h2 * 8:(h2 + 1) * 8],
                              xt_d[b, :, h2 * 8:(h2 + 1) * 8],
                          )
                      for h2 in range(2):
                          nc.sync.dma_start(
                              xp_sb[:, h2 * 8:(h2 + 1) * 8],
                              xp_d[b, :, h2 * 8:(h2 + 1) * 8],
                          )
                  else:
                      nc.sync.dma_start(xt_sb[:, 0, 0, 0:4], xt_d[b, :, 0, 0, 0:4])
                      nc.sync.dma_start(xp_sb[:, 0, 0:4], xp_d[b, :, 0, 0:4])
                  if b == NB - 1:
                      if dma:
                          for j in range(NH // 4):
                              nc.sync.dma_start(
                                  wv_sb[:, 2 * j:2 * j + 2], wv_d[:, 2 * j:2 * j + 2]
                              )
                          for i in range(KT // 2):
                              nc.sync.dma_start(
                                  wo_sb[:, 2 * i:2 * i + 2], wo_d[:, 2 * i:2 * i + 2]
                              )
                      else:
                          nc.sync.dma_start(wv_sb[:, 0, 0, 0, 0:4], wv_d[:, 0, 0, 0, 0:4])
                          nc.sync.dma_start(wo_sb[:, 0, 0, 0:4], wo_d[:, 0, 0, 0:4])

                  if not compute:
                      continue
                  # --- scores + exp: e[t,h] = exp(escale * x @ Wq) -----------
                  # exp reads the PSUM accumulator directly (ACT PSUM port is
                  # faster than its SBUF port; saves a DVE staging copy)
                  e_sb = epool.tile([128, MT, NH], bf16)
                  for m2 in range(MT // 4):
                      sc_ps = scps.tile([128, 4, NH], f32, tag="sc")
                      for m4 in range(4):
                          m = m2 * 4 + m4
                          for k in range(KT):
                              nc.tensor.matmul(
                                  sc_ps[:, m4, :],
                                  xt_sb[:, m, k, :],
                                  wq_sb[:, k, :],
                                  start=(k == 0),
                                  stop=(k == KT - 1),
                              )
                      nc.scalar.activation(
                          e_sb[:, m2 * 4:(m2 + 1) * 4, :],
                          sc_ps[:],
                          mybir.ActivationFunctionType.Exp,
                          scale=escale_sb[:],
                      )

                  # --- Z[h,1] = sum_t e*mask; 1/Z as per-partition scalar ----
                  z_ps = tps.tile([NH, 1], f32, tag="tps")
                  for m in range(MT):
                      nc.tensor.matmul(
                          z_ps[:],
                          e_sb[:, m, :],
                          mcol_sb[:, b, m:m + 1],
                          start=(m == 0),
                          stop=(m == MT - 1),
                      )
                  zinv_sb = work.tile([NH, 1], f32)
                  nc.vector.reciprocal(zinv_sb[:], z_ps[:])

                  # --- px[h, c] = e.T @ x; m-outer so pooling consumes each
                  # arriving xp chunk fully before the next is needed ---------
                  px_ps = pxps.tile([NH, 2, 512], f32, tag="px")
                  for m in range(MT):
                      for c2 in range(2):
                          nc.tensor.matmul(
                              px_ps[:, c2, :],
                              e_sb[:, m, :],
                              xp_sb[:, m, c2 * 512:(c2 + 1) * 512],
                              start=(m == 0),
                              stop=(m == MT - 1),
                          )
                  px_sb = pxsb.tile([NH, HIDDEN], bf16)
                  nc.vector.tensor_scalar_mul(
                      px_sb[:].rearrange("h (p c) -> h p c", p=2), px_ps[:],
                      zinv_sb[:],
                  )

                  # --- pxT: [C-tile, h] with b packed in the free dim --------
                  for k in range(KT):
                      pxt_ps = tps.tile([128, NH], bf16, tag="tps")
                      nc.tensor.transpose(
                          pxt_ps[:], px_sb[:, k * 128:(k + 1) * 128], idf_sb[:]
                      )
                      nc.vector.tensor_copy(pxall_sb[:, k, :, b], pxt_ps[:])

              if compute:
                  # --- stage 3 (transposed) interleaved with stage 4 ---------
                  # head h -> partition (h%2)*64, free group h//2: exactly the
                  # [128, KT, NB] layout stage 4 consumes. PSUM accumulation
                  # groups sharing a bank must not interleave, so each head
                  # pair j finishes its full k-contraction before the next;
                  # stage 4's j-th step then trails pair by pair.
                  o1t_ps = o1ps.tile([128, KT, NB], f32)
                  o1t_sb = opool.tile([128, KT, NB], bf16)
                  for j in range(KT):
                      for hh in range(2):
                          h = 2 * j + hh
                          lo = hh * 64
                          for k in range(KT):
                              nc.tensor.matmul(
                                  o1t_ps[lo:lo + 64, j, :],
                                  wv_sb[:, j, k, hh, :],
                                  pxall_sb[:, k, h, :],
                                  start=(k == 0),
                                  stop=(k == KT - 1),
                              )
                      nc.vector.tensor_copy(o1t_sb[:, j, :], o1t_ps[:, j, :])
                      for p2 in range(2):
                          nc.tensor.matmul(
                              of_ps[:, p2, :],
                              o1t_sb[:, j, :],
                              wo_sb[:, j, p2, :],
                              start=False,
                              stop=(j == KT - 1),
                          )
                  of_sb = opool.tile([NB, PROJ], f32)
                  nc.vector.tensor_copy(of_sb[:, 0:512], of_ps[:, 0, :])
                  nc.vector.tensor_copy(of_sb[:, 512:1024], of_ps[:, 1, :])
                  nc.sync.dma_start(out_d[:], of_sb[:])
              else:
                  nc.sync.dma_start(out_d[:], biasrep_sb[:])

    nc.compile()
    return nc


def _get_nc():
    global _CACHED_NC
    if _CACHED_NC is None:
        _CACHED_NC = _build_nc()
    return _CACHED_NC


def _prep_inputs(hidden_states, mask, kv_w, kv_b, out_w, out_b, query):
    """Host-side sharding + weight preprocessing -> per-core input maps."""
    x = np.ascontiguousarray(hidden_states, dtype=np.float32)
    mask = np.asarray(mask)
    kv_w = np.asarray(kv_w, dtype=np.float32)
    kv_b = np.asarray(kv_b, dtype=np.float32)
    out_w = np.asarray(out_w, dtype=np.float32)
    out_b = np.asarray(out_b, dtype=np.float32)
    query = np.asarray(query, dtype=np.float32)

    scale = 1.0 / HD ** 0.5
    Wk = kv_w[:, :HIDDEN]
    Wv = kv_w[:, HIDDEN:]
    qh = query.reshape(NH, HD)
    # fold query into the k-projection: Wq[c, h]
    Wq = np.einsum("chd,hd->ch", Wk.reshape(HIDDEN, NH, HD), qh) * scale
    bias_final = kv_b[HIDDEN:] @ out_w + out_b  # v-bias is exact post-pool

    # dynamic power-of-2 fp8 scales (exactly unwound inside the exp activation)
    sw = 2.0 ** np.floor(np.log2(F8MAX / max(np.abs(Wq).max(), 1e-30)))
    sx = 2.0 ** np.floor(np.log2(F8MAX / max(np.abs(x).max(), 1e-30)))
    sx = min(sx, 1.0)
    escale = np.full((128, 1), 1.0 / (sw * sx), np.float32)
    wq_r = np.ascontiguousarray(
        (Wq * sw).reshape(KT, 128, NH).transpose(1, 0, 2)
    ).astype(F8)  # [128, KT, NH], fp8 with exp-unwound scale
    wv_r = np.ascontiguousarray(
        Wv.reshape(KT, 128, NH // 2, 2, HD).transpose(1, 2, 0, 3, 4)
    ).astype(BF16)  # [128, pair, KT, 2, HD]
    wo_r = np.ascontiguousarray(
        out_w.reshape(KT, 128, 2, 512).transpose(1, 0, 2, 3)
    ).astype(BF16)  # [128, KT, 2, 512]
    idf = np.eye(NH, dtype=BF16)

    mvalid = (mask != 0).astype(np.float32)      # reference masks where mask == 0
    # xp[b, p, m, c] = (x*mask)[b, m*128+p, c]: per-partition contiguous 32KB
    xp_bf = np.ascontiguousarray(
        (x * mvalid[:, :, None]).reshape(B, MT, 128, HIDDEN).transpose(0, 2, 1, 3)
    ).astype(BF16)
    # xt[b, p, m, k, t] = x[b, m*128+t, k*128+p]: per-partition contiguous 16KB
    xt_bf = np.ascontiguousarray(
        (x * sx).reshape(B, MT, 128, KT, 128).transpose(0, 4, 1, 3, 2)
    ).astype(F8)

    in_maps = []
    for c in range(NCORES):
        sl = slice(c * NB, (c + 1) * NB)
        # mcol[p, b, m] = valid(mask[b, m*128+p])
        mcol = np.ascontiguousarray(
            mvalid[sl].reshape(NB, MT, 128).transpose(2, 0, 1)
        ).astype(BF16)
        in_maps.append({
            "xp": xp_bf[sl],
            "xt": xt_bf[sl],
            "wq": wq_r,
            "wv": wv_r,
            "wo": wo_r,
            "mcol": mcol,
            "biasrep": np.ascontiguousarray(
                np.broadcast_to(bias_final[None, :], (NB, PROJ))
            ),
            "idf": idf,
            "escale": escale,
        })
    return in_maps


def kernel(hidden_states, mask, kv_w, kv_b, out_w, out_b, query, **_unused):
    from concourse.bass_utils import run_bass_kernel_spmd

    nc = _get_nc()
    in_maps = _prep_inputs(hidden_states, mask, kv_w, kv_b, out_w, out_b, query)
    res = run_bass_kernel_spmd(nc, in_maps, list(range(NCORES)))
    out = np.concatenate([res.results[i]["out"] for i in range(NCORES)], axis=0)
    return out.astype(np.float32)
